# revision 25
# baseline (speedup 1.0000x reference)
"""Trainium2 Bass kernel for AttentionMLP (nn_AttentionMLP_72997264163220).

Reference computation:
  k/q/v = x @ W{k,q,v}.T + b      (D=3800 -> D)
  scores = q @ k.T / sqrt(D); attn = softmax(scores, -1)
  attended = attn @ v; h = attended.mean(seq)
  h = sigmoid(h @ W1.T + b1); h = sigmoid(h @ W2.T + b2); out = h @ W3.T + b3

Algebraic simplifications (host-side weight folding):
  1. scores = x' M x'^T with M = Wq'^T Wk' / sqrt(D) precomputed on host
     (x' carries a unit bias feature at d=3800), halving the projection
     matmul work vs computing q and k separately.
  2. The mean over the sequence commutes with attention and the (linear)
     v projection AND the first MLP layer's pre-activation:
       z1 = W1 (Wv^T (abar @ x) + bv) + b1 = W1v (abar @ x) + b1v,
     W1v = W1 Wv, b1v = W1 bv + b1 folded on host. Wv never reaches the
     device: the 3800x3800 v-projection collapses into a 512x3800 matmul
     against a single [D]-vector per batch.

Sharding: data-parallel over batch. 16 batches -> 8 cores x 2 batches
(512 tokens per core). All weights replicated. Big matmuls in fp8
DoubleRow (fp32 PSUM accumulate); softmax in fp32/bf16.

Device dataflow per core (SBUF partition dim first; D padded to
3840 = 30*128 with the bias feature at d=3800):
  xT    [128, 15, 2, 512] fp8  x^T (dp, pair, ko, token); row 3800 == 1
  x_tok [128, 2, 2, 3840] fp8  x (tok_p, tpair, tko, d); col 3800 == 1
  per d2-tile: t1[d2t] = M-pair^T x8 (DoubleRow over 15 pairs) -> fp8
  scores[2b+it] psum [128,256] += t1-pair-slice^T @ x8-pair  over pairs
  softmax rows (fp32 ACT/DVE) -> attn bf16; abar = colsum/S (x16) fp8
  xa8T[d-chunk] = x_tok-pair^T @ abar  (DoubleRow, 2 t-pairs)  fp8
  z1[ot] = w1v8-pair^T @ xa8T (DoubleRow, 15 d-pairs); sigmoid scale
  fp32 layers 2-3; biases via the unit feature / e0b unit rows.
DMA: m8 (14.7MB) streams on the SP queue; all other tensors go through
the Activation-engine queue so the M stream is never starved.
"""

import sys
import types

import numpy as np

if "/opt/trn_rl_repo" not in sys.path:
    sys.path.insert(0, "/opt/trn_rl_repo")


# ---------------------------------------------------------------------------
# NTFF profile hook shim (antenv.axon_hooks is absent in this image). Needed
# only when profiling (trace=True); harmless otherwise.
# ---------------------------------------------------------------------------
def _install_ntff_hook():
    try:
        import antenv  # noqa: F401

        if "antenv.axon_hooks" in sys.modules:
            return
        hooks_mod = types.ModuleType("antenv.axon_hooks")
        hooks_mod._hook = None

        def set_axon_ntff_profile_hook(h):
            hooks_mod._hook = h

        def get_axon_ntff_profile_hook():
            return hooks_mod._hook

        hooks_mod.set_axon_ntff_profile_hook = set_axon_ntff_profile_hook
        hooks_mod.get_axon_ntff_profile_hook = get_axon_ntff_profile_hook
        sys.modules["antenv.axon_hooks"] = hooks_mod
        import antenv as _a

        _a.axon_hooks = hooks_mod
        from trn_agent_boot.trn_boot import _ntff_profile_via_ctypes

        set_axon_ntff_profile_hook(
            _ntff_profile_via_ctypes("/opt/axon/libaxon_pjrt.so")
        )
    except Exception:
        pass


_install_ntff_hook()


def _install_verbose_cc_hook():
    """Wrap the PJRT->python compile callback so real tracebacks surface
    instead of an opaque 'CallFunctionObjArgs' error."""
    try:
        import traceback

        from concourse import bass2jax

        bass2jax.install_neuronx_cc_hook()
        import libneuronxla

        if getattr(libneuronxla, "_ant_verbose_wrap", False):
            return
        orig = libneuronxla.neuronx_cc

        def wrapped(*a, **k):
            try:
                return orig(*a, **k)
            except BaseException:
                traceback.print_exc()
                sys.stderr.flush()
                raise

        libneuronxla.neuronx_cc = wrapped
        libneuronxla._ant_verbose_wrap = True
        bass2jax.install_neuronx_cc_hook = lambda: None
    except Exception:
        pass


import bass_rust
import ml_dtypes
import concourse.bass as bass
import concourse.tile as tile
from concourse import mybir
from concourse.bass_utils import run_bass_kernel_spmd
from concourse.vector_clock import ScopedClock

BF16 = ml_dtypes.bfloat16

N_CORES = 8
B = 16  # batches total
S = 256  # seq len
D = 3800  # feature dim
H = 512  # hidden
C = 10  # classes

BLOC = B // N_CORES  # batches per core = 2
T = BLOC * S  # tokens per core = 512
DP = 3840  # D padded (+1 bias feature, up to 30*128)
KC = DP // 128  # 30 contraction chunks
ET = DP // 128  # 30 e-tiles of 128
PAIRS = KC // 2  # 15 DoubleRow chunk pairs
F32 = mybir.dt.float32
BF = mybir.dt.bfloat16
F16 = mybir.dt.float16
F8 = mybir.dt.float8e4
F8NP = mybir.dt.np(F8)  # ml_dtypes.float8_e4m3
# fp8 scale factors: weights are ~U(+-1/sqrt(3800)) which lands in e4m3's
# subnormal range, so weights are scaled up and the product scales are
# folded back out downstream (softmax scale / sigmoid activation scale).
XASCALE = 16.0  # on abar (via the ones vector), so xa fits e4m3 nicely
SC_SCALE = 4096.0  # on M = Wq^T Wk / sqrt(D); scores' = 4096 * scores
W1VSCALE = 256.0  # on W1v = W1 @ Wv
Z1_SCALE = XASCALE * W1VSCALE  # z1' = 4096 * z1


class SplitDrainTileContext(tile.TileContext):
    """This walrus build rejects >1 sync-wait on the tail Drain; split the
    global-clock waits across a chain of single-wait drain instructions."""

    MAXW = 1

    def _drain_and_barrier(self, tick_clock, wait_clock):
        nc = self.nc
        drain_inst = nc.sync.drain()
        wait_clock.add_sem_waits(
            drain_inst.ins, ScopedClock({None: tick_clock.global_clock})
        )
        si = drain_inst.ins.sync_info
        if si is not None and si.on_wait and len(si.on_wait) > self.MAXW:
            waits = list(si.on_wait)
            si.on_wait = waits[: self.MAXW]
            rest = waits[self.MAXW :]
            for i in range(0, len(rest), self.MAXW):
                extra = nc.sync.drain()
                extra.ins.sync_info = bass_rust.SyncInfo(
                    on_wait=rest[i : i + self.MAXW], on_update=[]
                )
        nc.all_engine_barrier()
        assert self.sems is not None
        popped = nc._tile_sem_poison_stack.pop()
        assert popped is self._sem_poison
        nc.clear_and_free_semaphores(list(self.sems.allocated().values()))
        nc.all_engine_barrier()


def _fix_excess_waits(nc, aux_sem, maxw=1):
    """Walrus in this image rejects instructions with more than ~1 sync
    wait. Compute-engine instructions: hoist extra waits onto same-engine
    no-ops inserted just before (sequencers execute in order). DMACopy:
    its waits live in the DGE queue descriptor, so an SP-side chain waits
    on all the original conditions, bumps `aux_sem`, and the descriptor
    waits on aux_sem alone."""
    aux_count = 0
    for f in nc.m.functions:
        for bb in f.blocks:
            insts = bb.instructions
            if not any(
                i.sync_info and i.sync_info.on_wait
                and len(i.sync_info.on_wait) > maxw
                for i in insts
            ):
                continue
            out = []
            for ins in insts:
                si = ins.sync_info
                nw = len(si.on_wait) if si and si.on_wait else 0
                if nw > maxw:
                    waits = list(si.on_wait)
                    if isinstance(ins, mybir.InstDMACopy):
                        for j, w in enumerate(waits):
                            nop = mybir.InstNoOp(name=f"{ins.name}-dw{j}")
                            nop.engine = mybir.EngineType.SP
                            nop.sync_info = bass_rust.SyncInfo(
                                on_wait=[w], on_update=[]
                            )
                            out.append(nop)
                        aux_count += 1
                        inc = mybir.InstNoOp(name=f"{ins.name}-dinc")
                        inc.engine = mybir.EngineType.SP
                        inc.sync_info = bass_rust.SyncInfo(
                            on_wait=[],
                            on_update=[
                                bass_rust.SyncUpdate(
                                    sync_type="semaphore",
                                    id=aux_sem.num,
                                    ant_name=aux_sem.name,
                                    update_mode="sem-add-imm",
                                    update_value=1,
                                    update_reg=None,
                                )
                            ],
                        )
                        out.append(inc)
                        si.on_wait = [
                            bass_rust.SyncWait(
                                sync_type="semaphore",
                                id=aux_sem.num,
                                ant_name=aux_sem.name,
                                wait_mode="sem-ge-imm",
                                wait_value=aux_count,
                                wait_reg=None,
                            )
                        ]
                    else:
                        keep = waits[-maxw:]
                        rest = waits[:-maxw]
                        for j, w in enumerate(rest):
                            nop = mybir.InstNoOp(name=f"{ins.name}-xw{j}")
                            nop.engine = ins.engine
                            nop.sync_info = bass_rust.SyncInfo(
                                on_wait=[w], on_update=[]
                            )
                            out.append(nop)
                        si.on_wait = keep
                out.append(ins)
            bb.instructions = out
    if aux_count:
        # reset aux sem at the very end so a re-executed NEFF starts clean
        f = nc.m.functions[0]
        bb = list(f.blocks)[-1]
        rst = mybir.InstNoOp(name="auxwait-reset")
        rst.engine = mybir.EngineType.SP
        rst.sync_info = bass_rust.SyncInfo(
            on_wait=[],
            on_update=[
                bass_rust.SyncUpdate(
                    sync_type="semaphore",
                    id=aux_sem.num,
                    ant_name=aux_sem.name,
                    update_mode="sem-sub-imm",
                    update_value=aux_count,
                    update_reg=None,
                )
            ],
        )
        il = bb.instructions
        il.append(rst)
        bb.instructions = il


def build_kernel() -> bass.Bass:
    nc = bass.Bass()

    x_d = nc.declare_dram_parameter("x8", [128, PAIRS, 2, T], F8, isOutput=False)
    xtok_d = nc.declare_dram_parameter("xtok8", [128, 2, 2, DP], F8,
                                       isOutput=False)
    m8_d = nc.declare_dram_parameter("m8", [ET // 2, 128, 2, PAIRS, 2, 128],
                                     F8, isOutput=False)
    w1v_d = nc.declare_dram_parameter("w1v8", [128, PAIRS, 2, H], F8,
                                      isOutput=False)
    w2_d = nc.declare_dram_parameter("w2", [128, 5, H], F16, isOutput=False)
    w3_d = nc.declare_dram_parameter("w3", [128, 5, C], F16, isOutput=False)
    e0b_d = nc.declare_dram_parameter("e0b", [128, BLOC], F16, isOutput=False)
    idn_d = nc.declare_dram_parameter("idn", [2, 2], F16, isOutput=False)
    out_d = nc.declare_dram_parameter("out", [BLOC, C], F32, isOutput=True)

    aux_sem = nc.alloc_semaphore("auxwait")
    with SplitDrainTileContext(nc) as tc:
        with tc.tile_pool(name="persist", bufs=1) as persist:
            _emit(nc, tc, persist, x_d, xtok_d, m8_d, w1v_d, w2_d,
                  w3_d, e0b_d, idn_d, out_d)
    _fix_excess_waits(nc, aux_sem)
    return nc


def _emit(nc, tc, persist, x_d, xtok_d, m8_d, w1v_d, w2_d, w3_d,
          e0b_d, idn_d, out_d):
    # ------------------ persistent tiles ------------------
    # x8 in 2 group tiles: DMA throughput is packet-rate-bound, so big
    # per-partition lines (7-8KB) move ~4x faster than per-pair 1KB lines;
    # two groups still stagger the first matmuls' data arrival.
    XGS = (7, 8)
    XG0 = (0, 7)  # prefix offsets
    x8g = [persist.tile([128, gs, 2, T], F8, name=f"x8g{g}", tag=f"x8g{g}")
           for g, gs in enumerate(XGS)]

    def x8c(p):
        g = 0 if p < 7 else 1
        return x8g[g][:, p - XG0[g]]

    # abar, fp8, DR layout per tpair [tok_p, tko, pad16]; only col b used.
    # Split per batch so phase 3's tp=0 matmuls start as soon as batch 0's
    # softmax lands, overlapping batch 1's softmax chain.
    ab8 = [persist.tile([128, 2, 16], F8, name=f"ab8{b}", tag=f"ab8{b}")
           for b in range(2)]
    nc.vector.memset(ab8[0][:], 0.0)
    nc.vector.memset(ab8[1][:], 0.0)
    # 2x2 identity for PE transposes in the MLP tail (DMA-ed constant:
    # memset cannot write at a nonzero partition offset)
    ident = persist.tile([2, 2], F16)
    x_tok8 = persist.tile([128, 2, 2, DP], F8)
    xa8T = persist.tile([128, PAIRS, 2, 16], F8)
    # t1 = (M8^T x8): fp8, [d2 within tile, d2-tile, token]
    t1_sb = persist.tile([128, KC, T], F8)

    # MLP weights: tiles up-front, DMAs issued a few iterations into
    # phase 1 so they overlap compute instead of the critical startup
    mlpw = tc.alloc_tile_pool(name="mlpw", bufs=1)
    w1v_t = mlpw.tile([128, PAIRS, 2, H], F8)
    w2_t = mlpw.tile([128, 5, H], F16)
    w3_t = mlpw.tile([128, 5, C], F16)
    e0b_t = mlpw.tile([128, BLOC], F16)

    # ---- phase 1a: t1 = M^T x  (scores = x M x^T = t1^T x, M = Wq^T Wk) ----
    DR = mybir.MatmulPerfMode.DoubleRow
    with tc.tile_pool(name="psum_sc", bufs=1, space="PSUM") as psum_sc:
        ps = [
            psum_sc.tile([128, S], F32, name=f"scores{i}", tag=f"scores{i}")
            for i in range(4)  # index = 2*b + it
        ]
        with (
            tc.tile_pool(name="mpool", bufs=1) as mpool,
            tc.tile_pool(name="psum_kq", bufs=1, space="PSUM") as psum_kq,
        ):
            m_t2 = None
            for d2t in range(ET):
                if d2t == 0:
                    # First-needed data split across BOTH queues: x8g0 and
                    # half of m8 pairchunk 0 lead the fast-starting SP
                    # queue; the other half and x8g1 lead the ACT queue
                    # (which has a ~3.8us first-use lag). Later pairchunks
                    # stream on SP as single big-line DMAs.
                    m_t2 = mpool.tile([128, 2, PAIRS, 2, 128], F8,
                                      tag="m8", bufs=3)
                    nc.sync.dma_start(x8g[0][:], x_d[:, 0 : XGS[0]])
                    nc.sync.dma_start(m_t2[:, 0], m8_d[0, :, 0])
                    nc.scalar.dma_start(m_t2[:, 1], m8_d[0, :, 1])
                    nc.scalar.dma_start(x8g[1][:], x_d[:, XGS[0] :])
                elif d2t % 2 == 0:
                    # paired chunks: 7680B per-partition lines stream at
                    # ~2x the rate of single-chunk 3840B lines
                    m_t2 = mpool.tile([128, 2, PAIRS, 2, 128], F8,
                                      tag="m8", bufs=3)
                    nc.sync.dma_start(m_t2[:], m8_d[d2t // 2])
                m_t = m_t2[:, d2t % 2]
                # Prefetches are gated behind dummy copies that read a
                # later t1 chunk: the WAW dependency on the destination
                # tile delays each transfer past the DMA-critical ramp,
                # so the m8 stream and x8 own the fabric early on.
                if d2t == 4:
                    nc.gpsimd.tensor_copy(x_tok8[0:1, 0, 0, 0:1],
                                          t1_sb[0:1, 3, 0:1])
                    nc.scalar.dma_start(x_tok8[:], xtok_d[:])
                if d2t == 5:
                    nc.gpsimd.tensor_copy(w2_t[0:1, 0, 0:1],
                                          t1_sb[0:1, 4, 0:1])
                    nc.scalar.dma_start(w2_t[:], w2_d[:])
                    nc.scalar.dma_start(w3_t[:], w3_d[:])
                    nc.scalar.dma_start(e0b_t[:], e0b_d[:])
                    nc.scalar.dma_start(ident[:], idn_d[:])
                if d2t == 8:
                    nc.gpsimd.tensor_copy(w1v_t[0:1, 0, 0, 0:1],
                                          t1_sb[0:1, 7, 0:1])
                    nc.scalar.dma_start(w1v_t[:], w1v_d[:])

                pt = psum_kq.tile([128, T], F32, tag="pt", bufs=2)
                for p in range(PAIRS):
                    nc.tensor.matmul(
                        pt[:], m_t[:, p], x8c(p),
                        start=(p == 0), stop=(p == PAIRS - 1),
                        perf_mode=DR,
                    )
                nc.vector.tensor_copy(t1_sb[:, d2t, :], pt[:])

            # ---- phase 1b: scores'[i, j] = sum_d2 t1[d2, i] x8[d2, j].
            # Kept as a separate block: the per-tile staggered completion
            # lets the softmax chains overlap the remaining 1b matmuls.
            for b in range(BLOC):
                for it in range(2):
                    i0 = b * S + it * 128
                    for p in range(PAIRS):
                        nc.tensor.matmul(
                            ps[2 * b + it][:],
                            t1_sb[:, 2 * p : 2 * p + 2, i0 : i0 + 128],
                            x8c(p)[:, :, b * S : (b + 1) * S],
                            start=(p == 0), stop=(p == PAIRS - 1),
                            perf_mode=DR,
                        )

        # ---- phase 2+3: softmax, abar, and xa, interleaved per batch ----
        # scores/SC_SCALE are ~N(0,1) so exp() is safe in fp32 without the
        # max subtraction; the row normalization (1/rowsum * XASCALE/S) is
        # folded into the moving vector of the column-sum matmul. The PE is
        # in-order, so batch 0's xa sweep (tp=0) is EMITTED between the two
        # batches' softmaxes: it runs while ACT/DVE chew on batch 1. All 30
        # xa accumulators live in one PSUM bank; one bulk cast writes xa8T.
        with (
            tc.tile_pool(name="smx", bufs=1) as smx,
            tc.tile_pool(name="psum_ab", bufs=1, space="PSUM") as psum_ab,
            tc.tile_pool(name="psum_xa", bufs=1, space="PSUM") as psum_xa,
        ):
            pab = psum_ab.tile([128, 4, 1], F32, name="pab")
            pxa = psum_xa.tile([128, PAIRS, 2, BLOC], F32, tag="pxa")
            for b in range(BLOC):
                for it in range(2):
                    p = ps[2 * b + it]
                    pexp = smx.tile([128, S], BF, tag="pexp", bufs=2)
                    sm = smx.tile([128, 1], F32, tag="sm", bufs=2)
                    nc.scalar.activation(
                        pexp[:], p[:], mybir.ActivationFunctionType.Exp,
                        scale=1.0 / SC_SCALE, accum_out=sm[:],
                    )
                    rin = smx.tile([128, 1], F32, tag="rin", bufs=2)
                    nc.vector.reciprocal(rin[:], sm[:])
                    wnrm = smx.tile([128, 1], BF, tag="wnrm", bufs=2)
                    nc.vector.tensor_scalar_mul(wnrm[:], rin[:], XASCALE / S)
                    for jc in range(2):
                        nc.tensor.matmul(
                            pab[:, 2 * b + jc, :],
                            pexp[:, jc * 128 : (jc + 1) * 128],
                            wnrm[:],
                            start=(it == 0), stop=(it == 1),
                            skip_group_check=True,
                        )
                for jc in range(2):
                    # tt = 2b + jc -> ab8[tpair=b][:, tko=jc, col b]
                    nc.vector.tensor_copy(
                        ab8[b][:, jc, b : b + 1], pab[:, 2 * b + jc, :]
                    )
                for dt in range(KC):
                    nc.tensor.matmul(
                        pxa[:, dt // 2, dt % 2, :],
                        x_tok8[:, b, :, dt * 128 : (dt + 1) * 128],
                        ab8[b][:, :, 0:BLOC],
                        start=(b == 0), stop=(b == 1),
                        perf_mode=DR,
                        skip_group_check=True,
                    )
            xa_cp = nc.vector.tensor_copy(xa8T[:, :, :, 0:BLOC], pxa[:])

    # --------- phase 4: MLP, row-oriented ([batch, feature] on 2 psum
    # partitions): z = xa^T-chunk-stationary (2-col LDW, ~free) with the
    # weight matrix as the moving stream; PE transposes flip the sigmoid
    # outputs back to feature-partition layout between layers.
    with (
        tc.tile_pool(name="mlph", bufs=1) as mlph,
        tc.tile_pool(name="psum_m", bufs=1, space="PSUM") as psum_m,
    ):
        # z1 rows [b=2, o=512] accumulated over 15 DoubleRow d-pairs
        pz1 = psum_m.tile([BLOC, H], F32, tag="pz1")
        for p in range(PAIRS):
            nc.tensor.matmul(
                pz1[:],
                xa8T[:, p, :, 0:BLOC],
                w1v_t[:, p, :, :],
                start=(p == 0), stop=(p == PAIRS - 1),
                perf_mode=DR,
            )
        # sigmoid in 128-wide chunks, each immediately transposed, so the
        # serial ACT latency overlaps the PE transposes
        h1r = mlph.tile([BLOC, H], F16, name="h1r")
        h1T = mlph.tile([128, 5, BLOC], F16, name="h1T")
        nc.vector.tensor_copy(h1T[:, 4, :], e0b_t[:])
        for oc in range(4):
            nc.scalar.activation(
                h1r[:, oc * 128 : (oc + 1) * 128],
                pz1[:, oc * 128 : (oc + 1) * 128],
                mybir.ActivationFunctionType.Sigmoid,
                scale=1.0 / Z1_SCALE,
            )
            ptr = psum_m.tile([128, BLOC], F16, tag="ptr", bufs=2)
            nc.tensor.transpose(
                ptr[:], h1r[:, oc * 128 : (oc + 1) * 128], ident[:]
            )
            nc.vector.tensor_copy(h1T[:, oc, :], ptr[:])

        pz2 = psum_m.tile([BLOC, H], F32, tag="pz2")
        for oc in range(5):
            nc.tensor.matmul(
                pz2[:], h1T[:, oc, :], w2_t[:, oc, :],
                start=(oc == 0), stop=(oc == 4),
            )
        h2r = mlph.tile([BLOC, H], F16, name="h2r")
        h2T = mlph.tile([128, 5, BLOC], F16, name="h2T")
        nc.vector.tensor_copy(h2T[:, 4, :], e0b_t[:])
        for oc in range(4):
            nc.scalar.activation(
                h2r[:, oc * 128 : (oc + 1) * 128],
                pz2[:, oc * 128 : (oc + 1) * 128],
                mybir.ActivationFunctionType.Sigmoid,
            )
            ptr = psum_m.tile([128, BLOC], F16, tag="ptr", bufs=2)
            nc.tensor.transpose(
                ptr[:], h2r[:, oc * 128 : (oc + 1) * 128], ident[:]
            )
            nc.vector.tensor_copy(h2T[:, oc, :], ptr[:])

        pz3 = psum_m.tile([BLOC, C], F32, tag="pz3")
        for oc in range(5):
            nc.tensor.matmul(
                pz3[:], h2T[:, oc, :], w3_t[:, oc, :],
                start=(oc == 0), stop=(oc == 4),
            )
        out_sb = mlph.tile([BLOC, C], F32)
        nc.vector.tensor_copy(out_sb[:], pz3[:])
        nc.scalar.dma_start(out_d[:], out_sb[:])
    mlpw.release()


# ---------------------------------------------------------------------------
# Host-side packing
# ---------------------------------------------------------------------------
def _f8(a):
    return np.clip(a, -240.0, 240.0).astype(F8NP)


def _pack_m8(Wq, bq, Wk, bk):
    """M = Wq'^T Wk' / sqrt(D), where W' carries its bias in column d=3800.
    scores = x' M x'^T reproduces q @ k.T / sqrt(D) exactly (the unit bias
    feature of x' supplies the bias cross terms). Scaled by SC_SCALE for
    e4m3 range, DoubleRow-interleaved to [ET, 128, PAIRS, 2, 128]:
    A[d2t, d1p, p, ko, d2p] = SC_SCALE * M[(2p+ko)*128+d1p, d2t*128+d2p]."""
    Wqp = np.zeros((D, DP), dtype=np.float32)
    Wqp[:, :D] = Wq
    Wqp[:, D] = bq
    Wkp = np.zeros((D, DP), dtype=np.float32)
    Wkp[:, :D] = Wk
    Wkp[:, D] = bk
    M = (Wqp.T @ Wkp) * np.float32(SC_SCALE / np.sqrt(np.float64(D)))
    A = M.reshape(PAIRS, 2, 128, ET, 128).transpose(3, 2, 0, 1, 4)
    # pair consecutive d2 chunks, partition-major: [15, 128, 2, P, 2, 128]
    A2 = A.reshape(ET // 2, 2, 128, PAIRS, 2, 128).transpose(0, 2, 1, 3, 4, 5)
    return np.ascontiguousarray(_f8(A2))


def _pack_w1v8(W1, b1, Wv, bv):
    """Fold the v-projection into the first MLP layer:
    W1v = W1 @ Wv [H, D], b1v = W1 @ bv + b1. Layout [128, PAIRS, 2, H]:
    A[dp, p, ko, o] = W1VSCALE * W1v[o, (2p+ko)*128+dp]; the bias lives in
    the d=3800 row against xa's unit feature (xa'[3800] == XASCALE, so the
    row carries b1v * W1VSCALE / XASCALE)."""
    W1v = (W1.astype(np.float64) @ Wv.astype(np.float64)).astype(np.float32)
    b1v = W1 @ bv + b1
    Wp = np.zeros((DP, H), dtype=np.float32)
    Wp[:D, :] = W1v.T * np.float32(W1VSCALE)
    Wp[D, :] = b1v * np.float32(W1VSCALE / XASCALE)
    A = Wp.reshape(PAIRS, 2, 128, H).transpose(2, 0, 1, 3)
    return np.ascontiguousarray(_f8(A))


def _pack_x8(xc):
    """xc [BLOC, S, D] -> [128, PAIRS, 2, T] e4m3, bias row d=3800 = 1."""
    xt = np.zeros((DP, T), dtype=np.float32)
    xt[:D, :] = xc.reshape(T, D).T
    xt[D, :] = 1.0
    A = xt.reshape(PAIRS, 2, 128, T).transpose(2, 0, 1, 3)
    return np.ascontiguousarray(_f8(A))


def _pack_xtok8(xc):
    """xc [BLOC, S, D] -> [128, 2, 2, DP] e4m3 (token partition, DoubleRow
    pairs of 128-token chunks), col d=3800 = 1."""
    xp = np.zeros((T, DP), dtype=np.float32)
    xp[:, :D] = xc.reshape(T, D)
    xp[:, D] = 1.0
    A = xp.reshape(2, 2, 128, DP).transpose(2, 0, 1, 3)
    return np.ascontiguousarray(_f8(A))


def _pack_w2(W2, b2):
    A = np.zeros((128, 5, H), dtype=np.float32)
    A[:, :4, :] = W2.T.reshape(4, 128, H).transpose(1, 0, 2)
    A[0, 4, :] = b2
    return np.ascontiguousarray(A, dtype=np.float16)


def _pack_w3(W3, b3):
    A = np.zeros((128, 5, C), dtype=np.float32)
    A[:, :4, :] = W3.T.reshape(4, 128, C).transpose(1, 0, 2)
    A[0, 4, :] = b3
    return np.ascontiguousarray(A, dtype=np.float16)


_NC_CACHE = {}


def _get_nc():
    if "nc" not in _NC_CACHE:
        _NC_CACHE["nc"] = build_kernel()
    return _NC_CACHE["nc"]


def kernel(x, Wk, bk, Wq, bq, Wv, bv, W1, b1, W2, b2, W3, b3, _trace=False):
    x = np.asarray(x, dtype=np.float32)

    m8_p = _pack_m8(
        np.asarray(Wq, np.float32), np.asarray(bq, np.float32),
        np.asarray(Wk, np.float32), np.asarray(bk, np.float32),
    )
    w1v_p = _pack_w1v8(
        np.asarray(W1, np.float32), np.asarray(b1, np.float32),
        np.asarray(Wv, np.float32), np.asarray(bv, np.float32),
    )
    w2_p = _pack_w2(np.asarray(W2, np.float32), np.asarray(b2, np.float32))
    w3_p = _pack_w3(np.asarray(W3, np.float32), np.asarray(b3, np.float32))
    e0b = np.zeros((128, BLOC), dtype=np.float16)
    e0b[0, :] = 1.0
    idn = np.eye(2, dtype=np.float16)

    in_maps = []
    for c in range(N_CORES):
        xc = x[c * BLOC : (c + 1) * BLOC]
        in_maps.append(
            {
                "x8": _pack_x8(xc),
                "xtok8": _pack_xtok8(xc),
                "m8": m8_p,
                "w1v8": w1v_p,
                "w2": w2_p,
                "w3": w3_p,
                "e0b": e0b,
                "idn": idn,
            }
        )

    nc = _get_nc()
    _install_verbose_cc_hook()
    res = run_bass_kernel_spmd(nc, in_maps, list(range(N_CORES)), trace=_trace)
    out = np.zeros((B, C), dtype=np.float32)
    for c in range(N_CORES):
        out[c * BLOC : (c + 1) * BLOC] = res.results[c]["out"]
    if _trace:
        return out, res
    return out
